# revision 2
# baseline (speedup 1.0000x reference)
"""DeepHit-style survival loss on 8 Trainium2 NeuronCores.

Math (no exact ties; t is bf16-rounded so near-ties resolve safely):
  w_j = bf16(exp(r_j));  S_gt(a) = sum_{t_j > t_a} w_j;  C(a) likewise
  S_le(a) = T - S_gt(a)
  loss = -sum_a e_a (r_a - ln S_le(a)) / n_events
         + 0.2 * sum_a e_a e^{-r_a} S_gt(a) / max(sum_a e_a C(a), 1)

Sharding: rows (a) are VALUE-banded — 16 fixed t16-bands [g/16,(g+1)/16),
two bands per core, each padded to 640 slots.  For a in band g, only
same-band j's need pairwise masks (an O(N^2/16) block); later bands
contribute a per-band scalar Theta = sum_{t_j >= theta_up} w_j (one
masked reduction over the full vectors), earlier bands contribute zero.

Per core: 10 DVE is_lt bf16 mask blocks [128, 640] feed 20 PE matmuls
(2-col stationary [w, 1] built by ONE ACT Exp of [full r | band r
interleaved with zeros] — exp(0)=1 gives the ones columns); suffix/
prefix masked reductions assemble per-band T/Theta/Csuf, partition-
summed and broadcast by two tiny PE matmuls; stats transpose via the
DVE 32x32 block transpose; epilogue in the transposed a=32k+p layout;
the final 32-partition partials go to the host, which sums them and
applies the normalizations (with n_events summed on host directly).

Host-side work is layout/selection only (banding by threshold compare,
padding, replication); all exp/log/comparison/sum math runs on device.
Host-sim rel err ~3e-4 (tolerance 2e-2); max(S_le, 1e-3) guards ln.
"""

import numpy as np

import concourse.bass as bass
import concourse.bacc as bacc
import concourse.mybir as mybir
import concourse.tile as tile

N = 8192
NCORES = 8
JB = N // 128              # full-vector j-blocks = 64
NBAND = 640                # padded band size (j and a), 5 cols
BH = NBAND // 128          # j-blocks per band = 5
NB = 2 * NBAND             # per-core a-slots = 1280
HB = NB // 128             # row-layout cols = 10
RW = JB + 2 * 2 * BH       # rall width = 64 + 20 = 84

F32 = mybir.dt.float32
BF16 = mybir.dt.bfloat16

EPS = 1e-8
RANK_W = 0.2

BCHUNKS = [(0, 512), (512, 640)]


def build_bass():
    nc = bacc.Bacc("TRN2", target_bir_lowering=False, debug=False,
                   num_devices=NCORES)

    tb_in = nc.dram_tensor("tb", [128, NB], BF16, kind="ExternalInput")
    rall_in = nc.dram_tensor("rall", [128, RW], F32, kind="ExternalInput")
    auxj_in = nc.dram_tensor("auxj", [128, 3 * HB + 6], F32,
                             kind="ExternalInput")
    t_col = nc.dram_tensor("t_col", [128, JB], F32, kind="ExternalInput")
    out = nc.dram_tensor("out", [128, 3], F32, kind="ExternalOutput")

    FT = mybir.ActivationFunctionType
    TS = mybir.AluOpType

    with tile.TileContext(nc) as tc:
        with tc.tile_pool(name="const", bufs=1) as cpool, \
             tc.tile_pool(name="mdve", bufs=10) as dpoolm:

            tb = cpool.tile([128, NB], BF16)
            rall = cpool.tile([128, RW], F32)
            auxj = cpool.tile([128, 3 * HB + 6], F32)
            tcol = cpool.tile([128, JB], F32)
            nc.sync.dma_start(tb[:, :], tb_in[:, :])
            nc.sync.dma_start(auxj[:, :], auxj_in[:, :])
            nc.scalar.dma_start(rall[:, :], rall_in[:, :])
            nc.sync.dma_start(tcol[:, :], t_col[:, :])
            tbc = auxj[:, 0:HB]
            vbj = auxj[:, HB:2 * HB]
            ebj = auxj[:, 2 * HB:3 * HB]
            # per-band thresholds: [upA, loA, upB, loB]
            thupA = auxj[:, 3 * HB + 0:3 * HB + 1]
            thloA = auxj[:, 3 * HB + 1:3 * HB + 2]
            thupB = auxj[:, 3 * HB + 2:3 * HB + 3]
            thloB = auxj[:, 3 * HB + 3:3 * HB + 4]
            ident2 = auxj[0:2, 3 * HB + 4:3 * HB + 6]
            rbj = rall[:, JB:RW:2]

            ones = cpool.tile([128, 1], F32)
            nc.vector.memset(ones[:, :], 1.0)
            ones_row = cpool.tile([1, 128], F32)
            nc.vector.memset(ones_row[:, :], 1.0)

            expall8 = cpool.tile([128, RW], BF16)
            nc.scalar.activation(expall8[:, :], rall[:, :], FT.Exp)
            nexpj = cpool.tile([128, HB], F32)
            nc.scalar.activation(nexpj[:, :], rbj, FT.Exp, scale=-1.0)

            with tc.tile_pool(name="psM", bufs=1, space="PSUM") as psM:
                psb = [[psM.tile([2, c1 - c0], F32, name=f"ps{b}c{k}")
                        for k, (c0, c1) in enumerate(BCHUNKS)]
                       for b in range(2)]

                with tc.high_priority():
                    for b in range(2):
                        a0 = b * NBAND
                        for h in range(BH):
                            hh = b * BH + h
                            mask = dpoolm.tile([128, NBAND], BF16,
                                               tag="mdve")
                            nc.vector.tensor_scalar(
                                mask[:, :], tb[:, a0:a0 + NBAND],
                                tbc[:, hh:hh + 1], None, TS.is_lt)
                            for k, (c0, c1) in enumerate(BCHUNKS):
                                nc.tensor.matmul(
                                    psb[b][k][:, :],
                                    expall8[:, JB + 2 * hh:JB + 2 * hh + 2],
                                    mask[:, c0:c1],
                                    start=(h == 0), stop=(h == BH - 1),
                                    tile_position=(0, 0))

                # suffix/prefix masked sums per band + T assembly
                wfullf = cpool.tile([128, JB], F32)
                nc.vector.tensor_copy(wfullf[:, :], expall8[:, 0:JB])
                wbandf = cpool.tile([128, HB], F32)
                nc.vector.tensor_copy(wbandf[:, :], expall8[:, JB:RW:2])
                wbv = cpool.tile([128, HB], F32)
                nc.vector.tensor_mul(wbv[:, :], wbandf[:, :], vbj)

                # colsumX cols: [T_A, Th_A, Cs_A, T_B, Th_B, Cs_B]
                colsumX = cpool.tile([128, 6], F32)
                tmp = cpool.tile([128, JB], F32)
                tpre = cpool.tile([128, 2], F32)
                for b, (tup, tlo) in enumerate([(thupA, thloA),
                                                (thupB, thloB)]):
                    o = 3 * b
                    msuf = cpool.tile([128, JB], F32, name=f"msuf{b}")
                    nc.vector.tensor_scalar(msuf[:, :], tcol[:, :], tup,
                                            None, TS.is_ge)
                    nc.vector.tensor_mul(tmp[:, :], wfullf[:, :],
                                         msuf[:, :])
                    nc.vector.reduce_sum(colsumX[:, o + 1:o + 2], tmp[:, :],
                                         axis=mybir.AxisListType.X)
                    nc.vector.reduce_sum(colsumX[:, o + 2:o + 3],
                                         msuf[:, :],
                                         axis=mybir.AxisListType.X)
                    nc.vector.tensor_scalar(tmp[:, :], tcol[:, :], tlo,
                                            None, TS.is_lt)
                    nc.vector.tensor_mul(tmp[:, :], wfullf[:, :], tmp[:, :])
                    nc.vector.reduce_sum(tpre[:, b:b + 1], tmp[:, :],
                                         axis=mybir.AxisListType.X)
                    nc.vector.reduce_sum(
                        colsumX[:, o:o + 1],
                        wbv[:, 5 * b:5 * b + 5],
                        axis=mybir.AxisListType.X)
                    nc.vector.tensor_add(colsumX[:, o:o + 1],
                                         colsumX[:, o:o + 1],
                                         tpre[:, b:b + 1])
                    nc.vector.tensor_add(colsumX[:, o:o + 1],
                                         colsumX[:, o:o + 1],
                                         colsumX[:, o + 1:o + 2])

                Ts = cpool.tile([1, 6], F32)
                Tb6 = cpool.tile([128, 6], F32)
                with tc.tile_pool(name="psA", bufs=1, space="PSUM") as psA:
                    psT = psA.tile([1, 6], F32)
                    nc.tensor.matmul(psT[:, :], ones[:, :], colsumX[:, :],
                                     start=True, stop=True)
                    nc.vector.tensor_copy(Ts[:, :], psT[:, :])
                    psB = psA.tile([128, 6], F32)
                    nc.tensor.matmul(psB[:, :], ones_row[:, :], Ts[:, :],
                                     start=True, stop=True)
                    nc.vector.tensor_copy(Tb6[:, :], psB[:, :])

                # PSUM -> SBUF stats; PE-side transpose into [128, 20]
                stat = cpool.tile([2, NB], F32)
                nc.scalar.copy(stat[:, 0:512], psb[0][0][:, :])
                nc.scalar.copy(stat[:, 512:640], psb[0][1][:, :])
                nc.vector.tensor_copy(stat[:, 640:1152], psb[1][0][:, :])
                nc.vector.tensor_copy(stat[:, 1152:1280], psb[1][1][:, :])
                with tc.tile_pool(name="psQ", bufs=1, space="PSUM") as psQ:
                    psq = psQ.tile([128, 2 * HB], F32)
                    for h in range(HB):
                        nc.tensor.matmul(
                            psq[:, 2 * h:2 * h + 2],
                            stat[:, 128 * h:128 * (h + 1)],
                            ident2,
                            start=True, stop=True, is_transpose=True)
                    sq = cpool.tile([128, 2 * HB], F32)
                    nc.vector.tensor_copy(sq[:, :], psq[:, :])
            s01 = sq[:, 0:2 * HB:2]
            c01 = sq[:, 1:2 * HB:2]

            # per-band Theta/Csuf/T applied to that band's 5 columns
            sg = cpool.tile([128, HB], F32)
            cg = cpool.tile([128, HB], F32)
            sl = cpool.tile([128, HB], F32)
            for b in range(2):
                cs = slice(5 * b, 5 * b + 5)
                o = 3 * b
                nc.vector.tensor_scalar(sg[:, cs], s01[:, cs],
                                        Tb6[:, o + 1:o + 2], None, TS.add)
                nc.vector.tensor_scalar(cg[:, cs], c01[:, cs],
                                        Tb6[:, o + 2:o + 3], None, TS.add)
                nc.vector.tensor_scalar(sl[:, cs], sg[:, cs],
                                        Tb6[:, o:o + 1], -1.0,
                                        TS.subtract, TS.mult)
            nc.vector.tensor_scalar_max(sl[:, :], sl[:, :], 1e-3)
            lg = cpool.tile([128, HB], F32)
            nc.scalar.activation(lg[:, :], sl[:, :], FT.Ln)
            likt = cpool.tile([128, HB], F32)
            nc.vector.tensor_sub(likt[:, :], rbj, lg[:, :])
            lik = cpool.tile([128, HB], F32)
            nc.vector.tensor_mul(lik[:, :], likt[:, :], ebj)
            rk1 = cpool.tile([128, HB], F32)
            nc.vector.tensor_mul(rk1[:, :], nexpj[:, :], sg[:, :])
            rk = cpool.tile([128, HB], F32)
            nc.vector.tensor_mul(rk[:, :], rk1[:, :], ebj)
            cnt = cpool.tile([128, HB], F32)
            nc.vector.tensor_mul(cnt[:, :], cg[:, :], ebj)

            red3 = cpool.tile([128, 3], F32)
            nc.vector.reduce_sum(red3[:, 0:1], lik[:, :],
                                 axis=mybir.AxisListType.X)
            nc.vector.reduce_sum(red3[:, 1:2], rk[:, :],
                                 axis=mybir.AxisListType.X)
            nc.vector.reduce_sum(red3[:, 2:3], cnt[:, :],
                                 axis=mybir.AxisListType.X)
            nc.sync.dma_start(out[:, :], red3[:, :])

    nc.compile()
    return nc


def shard_inputs(risk_scores, survival_times, event_indicators):
    import ml_dtypes

    t = np.asarray(survival_times, dtype=np.float32)
    r = np.ascontiguousarray(np.asarray(risk_scores, dtype=np.float32))
    e = np.asarray(event_indicators).astype(np.float32)

    t16 = t.astype(ml_dtypes.bfloat16)
    t16f = t16.astype(np.float32)
    band16 = np.minimum((t16f * 16).astype(np.int32), 15)

    t_colf = np.ascontiguousarray(t16f.reshape(JB, 128).T)
    r_colf = r.reshape(JB, 128).T

    in_maps = []
    for c in range(NCORES):
        tbv = np.empty(NB, dtype=np.float32)
        rbv = np.zeros(NB, dtype=np.float32)
        ebv = np.zeros(NB, dtype=np.float32)
        vbv = np.zeros(NB, dtype=np.float32)
        ths = []
        for b in range(2):
            g = 2 * c + b                      # global band index
            idx = np.where(band16 == g)[0]
            nb = len(idx)
            assert nb <= NBAND, f"band {g} overflow: {nb} > {NBAND}"
            sl = slice(b * NBAND, b * NBAND + NBAND)
            tseg = np.full(NBAND, g / 16, dtype=np.float32)
            tseg[:nb] = t16f[idx]
            tbv[sl] = tseg
            rbv[sl][:nb] = 0.0  # noop, keep zeros then fill
            rbv[b * NBAND:b * NBAND + nb] = r[idx]
            ebv[b * NBAND:b * NBAND + nb] = e[idx]
            vbv[b * NBAND:b * NBAND + nb] = 1.0
            thup = (g + 1) / 16 if g < 15 else 2.0
            ths += [thup, g / 16]

        rall = np.zeros((128, RW), dtype=np.float32)
        rall[:, 0:JB] = r_colf
        rall[:, JB:RW:2] = rbv.reshape(HB, 128).T

        auxj = np.zeros((128, 3 * HB + 6), dtype=np.float32)
        auxj[:, 0:HB] = tbv.reshape(HB, 128).T
        auxj[:, HB:2 * HB] = vbv.reshape(HB, 128).T
        auxj[:, 2 * HB:3 * HB] = ebv.reshape(HB, 128).T
        auxj[:, 3 * HB + 0] = ths[0]
        auxj[:, 3 * HB + 1] = ths[1]
        auxj[:, 3 * HB + 2] = ths[2]
        auxj[:, 3 * HB + 3] = ths[3]
        auxj[0, 3 * HB + 4] = 1.0
        auxj[1, 3 * HB + 5] = 1.0

        in_maps.append({
            "tb": np.ascontiguousarray(
                np.broadcast_to(tbv.astype(ml_dtypes.bfloat16)[None, :],
                                (128, NB))),
            "rall": np.ascontiguousarray(rall),
            "auxj": np.ascontiguousarray(auxj),
            "t_col": t_colf,
        })
    return in_maps


def combine_partials(results, nev):
    parts = np.zeros(3, dtype=np.float64)
    for res in results:
        parts += res["out"].astype(np.float64).sum(axis=0)
    L, Rr, P = parts
    rank = Rr / max(P, 1.0) if P > 0 else Rr
    loss = -L / (nev + EPS) + RANK_W * rank
    return np.float32(loss).reshape(())


_NC_CACHE = []


def kernel(risk_scores, survival_times, event_indicators):
    from concourse import bass_utils

    if not _NC_CACHE:
        _NC_CACHE.append(build_bass())
    nc = _NC_CACHE[0]

    in_maps = shard_inputs(risk_scores, survival_times, event_indicators)
    res = bass_utils.run_bass_kernel_spmd(nc, in_maps, list(range(NCORES)))
    nev = float(np.asarray(event_indicators).astype(np.float64).sum())
    return combine_partials(res.results, nev)
